# revision 29
# baseline (speedup 1.0000x reference)
"""Trainium2 Bass kernel for nn_AffectChannel (compress + GELU + 16-dim GRU scan).

Strategy (8 NeuronCores, data-parallel over batch, one batch element per core):
  Phase 1 (memory-bound): compressed = gelu(residual @ compress_w.T + b)
    - residual shard is pre-transposed on host -> fully coalesced DMA, fp32
      matmuls contract d on partitions, accumulate in PSUM.
  Phase 2: x_gates = compressed @ w_ih.T + biases, written in a "blocked"
    layout: partitions = 16 hidden lanes x 8 time-blocks (512 steps each).
  Phase 3: the sequential GRU scan is computed by Picard iteration: gates are
    evaluated from the previous trajectory estimate (fully parallel, 128-lane
    ops), then the diagonal blend recurrence h_t = z_t h_{t-1} + (1-z_t) n_t
    is solved EXACTLY with the DVE tensor_tensor_scan instruction (per-block
    prefix scans + an 8-block boundary chain via tiny PE gather/scatter
    matmuls).  ~24 sweeps converge to fp32 accuracy (contraction ~0.5/sweep).
"""
import json
import os

import numpy as np

B, S, D, C, H = 8, 4096, 2048, 64, 16
NB = 8           # time blocks
L = S // NB      # block length = 512
NCORES = 8
N_SWEEP = int(os.environ.get("AFFECT_N_SWEEP", "23"))
N_BF = int(os.environ.get("AFFECT_N_BF", "13"))


# --- walrus workaround: split multi-wait instructions ----------------------
def _split_multiwaits(d):
    n = 0
    uid = [0]
    for f in d.get("functions", []):
        for blk in f.get("blocks", []):
            out = []
            for ins in blk.get("instructions", []):
                si = ins.get("sync_info")
                waits = (si or {}).get("on_wait") or []
                if len(waits) > 1:
                    n += 1
                    for w in waits[:-1]:
                        uid[0] += 1
                        out.append({
                            "opcode": "EventSemaphore",
                            "name": f"{ins['name']}_wsplit{uid[0]}",
                            "engine": ins["engine"],
                            "ins": [], "outs": [],
                            "debug": ins.get("debug"),
                            "sync_info": {"on_wait": [w], "on_update": []},
                        })
                    si["on_wait"] = [waits[-1]]
                out.append(ins)
            blk["instructions"] = out
    return n


def _fix_bir_json(bir_json):
    if isinstance(bir_json, str):
        bir_json = bir_json.encode()
    d = json.loads(bir_json)
    if _split_multiwaits(d) == 0:
        return bir_json
    return json.dumps(d).encode()


_PATCHED = False


def _install_bir_fix():
    global _PATCHED
    if _PATCHED:
        return
    _PATCHED = True
    import concourse.bass_utils as bu
    import concourse.bass2jax as b2j

    orig = bu.compile_bir_kernel

    def patched(bir_json, tmpdir, neff_name="file.neff"):
        return orig(_fix_bir_json(bir_json), tmpdir, neff_name=neff_name)

    bu.compile_bir_kernel = patched
    b2j.compile_bir_kernel = patched


# --- kernel build ----------------------------------------------------------
def _build_nc():
    import concourse.bass as bass
    import concourse.mybir as mybir
    from concourse.tile import TileContext
    from concourse.tile import add_dep_helper

    F32 = mybir.dt.float32
    BF16 = mybir.dt.bfloat16
    AF = mybir.ActivationFunctionType
    OP = mybir.AluOpType
    AX = mybir.AxisListType

    nc = bass.Bass("TRN2", target_bir_lowering=False)

    resT = nc.dram_tensor("resT", [D, S], F32, kind="ExternalInput")
    # all constants packed into one tensor: [cw(1024) wr wz wn(128*3) wih2*(32*3)
    #  gmat(16) smat_pad(128) maskb selm(8+8) cols(1*5)] = 1669 cols
    consts = nc.dram_tensor("consts", [128, 1669], F32, kind="ExternalInput")
    wbf = nc.dram_tensor("wbf", [128, 384], mybir.dt.bfloat16, kind="ExternalInput")
    out = nc.dram_tensor("out", [128, L], F32, kind="ExternalOutput")

    NDC = D // 128  # 16 d-chunks

    with TileContext(nc) as tc:
        with tc.tile_pool(name="const", bufs=1) as cst, \
             tc.tile_pool(name="persist", bufs=1) as per:
            call = cst.tile([128, 1669], F32, tag="call")
            nc.sync.dma_start(call[:], consts.ap())
            cw_sb = call[:, 0:1024]
            wr_sb = call[:, 1024:1152]
            wz_sb = call[:, 1152:1280]
            wn_sb = call[:, 1280:1408]
            wih2r_sb = call[:, 1408:1440]
            wih2z_sb = call[:, 1440:1472]
            wih2n_sb = call[:, 1472:1504]
            g_sb = call[:, 1504:1520]
            s_sb = call[0:16, 1520:1648]
            mb_sb = call[:, 1648:1656]
            sel_sb = call[:, 1656:1664]
            cb2_sb = call[:, 1664:1665]
            brc_sb = call[:, 1665:1666]
            bzc_sb = call[:, 1666:1667]
            bnc_sb = call[:, 1667:1668]
            bhn_sb = call[:, 1668:1669]
            wbf_sb = cst.tile([128, 384], BF16, tag="wbf")
            nc.sync.dma_start(wbf_sb[:], wbf.ap())
            wrb_sb = wbf_sb[:, 0:128]
            wzb_sb = wbf_sb[:, 128:256]
            wnb_sb = wbf_sb[:, 256:384]
            ones_sb = cst.tile([128, L], F32, tag="ones")
            nc.vector.memset(ones_sb, 1.0)

            # chunk-pair layout: rows 0-63 = even s-chunks, 64-127 = odd
            comp2 = per.tile([128, S // 2], F32, tag="comp2")
            xrz_sb = per.tile([128, 2 * L], F32, tag="xrz")
            xn_sb = per.tile([128, L], F32, tag="xn")

            # ---- Phase 1: compress matmul + gelu -------------------------
            # s-chunk pairs run CONCURRENTLY on the PE via column tiling:
            # even chunk -> array col strips 0-63, odd chunk -> 64-127.
            # The [128, L] psum pair tile is already in comp2's layout.
            with tc.tile_pool(name="resp", bufs=4) as resp, \
                 tc.tile_pool(name="cpsum", bufs=1, space="PSUM") as cpsum:
                ctiles = [cpsum.tile([128, L], F32, tag=f"c{p}", name=f"c{p}") for p in range(4)]
                for dc in range(NDC):
                    rt = resp.tile([128, S], F32, tag="res")
                    # split each 2MiB row-chunk across both HWDGE rings
                    nc.sync.dma_start(
                        rt[:, 0:S // 2],
                        resT.ap()[dc * 128:(dc + 1) * 128, 0:S // 2])
                    nc.scalar.dma_start(
                        rt[:, S // 2:S],
                        resT.ap()[dc * 128:(dc + 1) * 128, S // 2:S])
                    for p in range(4):
                        nc.tensor.matmul(
                            ctiles[p][0:64, :],
                            cw_sb[:, dc * C:(dc + 1) * C],
                            rt[:, (2 * p) * L:(2 * p + 1) * L],
                            start=(dc == 0), stop=(dc == NDC - 1),
                            tile_position=(0, 0),
                        )
                        nc.tensor.matmul(
                            ctiles[p][64:128, :],
                            cw_sb[:, dc * C:(dc + 1) * C],
                            rt[:, (2 * p + 1) * L:(2 * p + 2) * L],
                            start=(dc == 0), stop=(dc == NDC - 1),
                            tile_position=(0, 64),
                        )
                for p in range(4):
                    nc.scalar.activation(
                        comp2[:, L * p:L * (p + 1)],
                        ctiles[p][:], AF.Gelu, bias=cb2_sb,
                    )

            # ---- Phase 2: x-gates directly into blocked layout -----------
            # lhsT = blockdiag([wihT_g, wihT_g]) over the chunk-pair rows of
            # comp2 -> out [32, L] at partition 32j = blocks 2j (rows 0-15)
            # and 2j+1 (rows 16-31).
            with tc.tile_pool(name="xpsum", bufs=1, space="PSUM") as xpsum:
                for g, (wt2, dst, bias) in enumerate([
                    (wih2r_sb, xrz_sb[:, 0:L], brc_sb),
                    (wih2z_sb, xrz_sb[:, L:2 * L], bzc_sb),
                    (wih2n_sb, xn_sb[:], bnc_sb),
                ]):
                    ps = xpsum.tile([128, L], F32, tag=f"xg{g}", name=f"xg{g}")
                    for j in range(4):
                        nc.tensor.matmul(
                            ps[32 * j:32 * j + 32, :], wt2[:],
                            comp2[:, j * L:(j + 1) * L],
                            start=True, stop=True,
                            tile_position=(0, 32 * j),
                        )
                    nc.scalar.activation(
                        dst, ps[:], AF.Identity, bias=bias[:, 0:1],
                    )

            # ---- Phase 3: Picard sweeps ----------------------------------
            # hs tiles hold the SHIFTED trajectory: hs[:, 0] = boundary
            # column (end of previous block = h_{t-1} for the block start),
            # hs[:, 1:L] = h[:, 0:L-1].  Gate matmuls then read hs directly.
            with tc.tile_pool(name="spsum", bufs=1, space="PSUM") as sps, \
                 tc.tile_pool(name="swp", bufs=1) as swp:
                rzps = sps.tile([128, 2 * L], F32, tag="rzps")
                wps = sps.tile([128, L], F32, tag="wps")
                ptps = sps.tile([16, NB], F32, tag="ptps")
                otps = sps.tile([16, NB], F32, tag="otps")
                ebps = sps.tile([128, NB], F32, tag="ebps")

                hs0 = swp.tile([128, L], F32, tag="hs0")
                hs1 = swp.tile([128, L], F32, tag="hs1")
                h_out = swp.tile([128, L], F32, tag="hout")
                s_t = swp.tile([128, 2 * L], F32, tag="st")
                rz = swp.tile([128, 2 * L], F32, tag="rz")
                u_t = swp.tile([128, L], F32, tag="ut")
                v_t = swp.tile([128, L], F32, tag="vt")
                n_t = swp.tile([128, L], F32, tag="nt")
                c_t = swp.tile([128, L], F32, tag="ct")
                P_t = swp.tile([128, L], F32, tag="Pt")
                O_t = swp.tile([128, L], F32, tag="Ot")
                pm = swp.tile([128, NB], F32, tag="pm")
                om = swp.tile([128, NB], F32, tag="om")
                ot_sb = swp.tile([16, NB], F32, tag="otsb")
                hb_sb = swp.tile([16, NB], F32, tag="hbsb")
                eb_sb = swp.tile([128, NB], F32, tag="ebsb")

                hsb0 = swp.tile([128, L], BF16, tag="hsb0")
                hsb1 = swp.tile([128, L], BF16, tag="hsb1")
                hbq = swp.tile([128, 1], F32, tag="hbq")

                nc.vector.memset(hsb0[:], 0.0)
                nc.vector.memset(hs0[:], 0.0)
                hb_cur, hb_nxt = hsb0, hsb1
                h_cur, h_nxt = hs0, hs1
                for k in range(N_SWEEP):
                    # first N_BF sweeps evaluate the gate matmuls in bf16
                    # (1 PE pass instead of fp32's 2); PSUM accumulation and
                    # everything downstream stays fp32.  The fp32 tail
                    # re-converges to the exact fp32 fixpoint.
                    bf_gates = k < N_BF
                    bf_write = k < N_BF - 1
                    if bf_gates:
                        gate_list = ((wrb_sb, rzps[:, 0:L]),
                                     (wzb_sb, rzps[:, L:2 * L]),
                                     (wnb_sb, wps[:]))
                        rhs = hb_cur
                    else:
                        gate_list = ((wr_sb, rzps[:, 0:L]),
                                     (wz_sb, rzps[:, L:2 * L]),
                                     (wn_sb, wps[:]))
                        rhs = h_cur
                    for wt, ps in gate_list:
                        nc.tensor.matmul(
                            ps[:], wt, rhs[:], start=True, stop=True)
                    nc.vector.tensor_tensor(s_t[:], rzps[:], xrz_sb[:], OP.add)
                    nc.scalar.activation(rz[:], s_t[:], AF.Sigmoid)
                    nc.vector.scalar_tensor_tensor(
                        u_t[:], wps[:], bhn_sb, rz[:, 0:L], OP.add, OP.mult)
                    v_bi = nc.vector.tensor_tensor(v_t[:], u_t[:], xn_sb[:], OP.add)
                    nc.scalar.activation(n_t[:], v_t[:], AF.Tanh)
                    # P-scan is off the critical chain; schedule it on DVE
                    # after v so it hides behind the tanh instead of
                    # delaying the u->v->tanh chain.
                    p_bi = nc.vector.tensor_tensor_scan(
                        P_t[:], rz[:, L:2 * L], ones_sb, 1.0, OP.mult, OP.mult)
                    add_dep_helper(v_bi.ins, p_bi.ins, sync=False,
                                   reason="P-scan after v on DVE")
                    nc.vector.scalar_tensor_tensor(
                        c_t[:], rz[:, L:2 * L], 1.0, n_t[:], OP.subtract, OP.mult)
                    nc.vector.tensor_tensor_scan(
                        O_t[:], rz[:, L:2 * L], c_t[:], 0.0, OP.mult, OP.subtract)
                    # boundary chain across the 8 blocks
                    nc.vector.tensor_scalar(
                        pm[:], mb_sb, P_t[:, L - 1:L], None, OP.mult)
                    nc.vector.tensor_scalar(
                        om[:], mb_sb, O_t[:, L - 1:L], None, OP.mult)
                    nc.tensor.matmul(ptps[:], g_sb, pm[:], start=True, stop=True)
                    nc.tensor.matmul(otps[:], g_sb, om[:], start=True, stop=True)
                    nc.vector.tensor_copy(ot_sb[:], otps[:])
                    nc.vector.tensor_tensor_scan(
                        hb_sb[:], ptps[:], ot_sb[:], 0.0, OP.mult, OP.add)
                    nc.tensor.matmul(ebps[:], s_sb, hb_sb[:], start=True, stop=True)
                    nc.vector.tensor_tensor(eb_sb[:], ebps[:], sel_sb, OP.mult)
                    # boundary column -> hs_nxt[:, 0], shifted combine -> 1:L
                    nc.vector.tensor_reduce(hbq[:], eb_sb[:], AX.X, OP.add)
                    if bf_write:
                        nc.vector.tensor_copy(hb_nxt[:, 0:1], hbq[:])
                        nc.vector.scalar_tensor_tensor(
                            hb_nxt[:, 1:L], P_t[:, 0:L - 1], hbq[:, 0:1],
                            O_t[:, 0:L - 1], OP.mult, OP.add)
                        hb_cur, hb_nxt = hb_nxt, hb_cur
                    else:
                        nc.vector.tensor_copy(h_nxt[:, 0:1], hbq[:])
                        nc.vector.scalar_tensor_tensor(
                            h_nxt[:, 1:L], P_t[:, 0:L - 1], hbq[:, 0:1],
                            O_t[:, 0:L - 1], OP.mult, OP.add)
                        if k == N_SWEEP - 1:
                            nc.vector.scalar_tensor_tensor(
                                h_out[:], P_t[:], hbq[:, 0:1], O_t[:],
                                OP.mult, OP.add)
                        h_cur, h_nxt = h_nxt, h_cur

                nc.sync.dma_start(out.ap(), h_out[:])

    return nc


_NC_CACHE = None


def kernel(residual, compress_w, compress_b, w_ih, w_hh, b_ih, b_hh):
    global _NC_CACHE
    _install_bir_fix()
    from concourse.bass_utils import run_bass_kernel_spmd

    f32 = np.float32
    residual = np.ascontiguousarray(residual, dtype=f32)
    compress_w = np.asarray(compress_w, dtype=f32)
    compress_b = np.asarray(compress_b, dtype=f32)
    w_ih = np.asarray(w_ih, dtype=f32)
    w_hh = np.asarray(w_hh, dtype=f32)
    b_ih = np.asarray(b_ih, dtype=f32)
    b_hh = np.asarray(b_hh, dtype=f32)

    # host-side shared weight prep (layout only)
    cwT = np.ascontiguousarray(compress_w.T)                      # [D, C]
    cw_tiles = np.ascontiguousarray(
        cwT.reshape(D // 128, 128, C).transpose(1, 0, 2).reshape(128, -1))
    wihT = np.ascontiguousarray(w_ih.T)                           # [C, 3H]

    def wih2(g):   # [128, 32] blockdiag over chunk-pair rows
        m = np.zeros((128, 32), f32)
        m[0:C, 0:16] = wihT[:, 16 * g:16 * g + 16]
        m[C:2 * C, 16:32] = wihT[:, 16 * g:16 * g + 16]
        return m

    def blockdiag_tiled(w):   # w: [H, H] -> [128, 128] (8 diagonal blocks)
        return np.ascontiguousarray(np.kron(np.eye(NB, dtype=f32), w.T.astype(f32)))

    wr_t = blockdiag_tiled(w_hh[:H])
    wz_t = blockdiag_tiled(w_hh[H:2 * H])
    wn_t = blockdiag_tiled(w_hh[2 * H:])

    g_np = np.zeros((128, 16), f32)
    for kk in range(128):
        g_np[kk, kk % 16] = 1.0
    s_np = np.ascontiguousarray(g_np.T)
    maskb_np = np.zeros((128, NB), f32)
    sel_np = np.zeros((128, NB), f32)
    for kk in range(128):
        maskb_np[kk, kk // 16] = 1.0
        if kk // 16 >= 1:
            sel_np[kk, kk // 16 - 1] = 1.0

    smat_pad = np.zeros((128, 128), f32)
    smat_pad[0:16, :] = s_np
    consts_np = np.concatenate([
        cw_tiles, wr_t, wz_t, wn_t, wih2(0), wih2(1), wih2(2), g_np, smat_pad,
        maskb_np, sel_np,
        np.tile(compress_b, 2).reshape(128, 1),
        np.tile(b_ih[:H] + b_hh[:H], NB).reshape(128, 1),
        np.tile(b_ih[H:2 * H] + b_hh[H:2 * H], NB).reshape(128, 1),
        np.tile(b_ih[2 * H:], NB).reshape(128, 1),
        np.tile(b_hh[2 * H:], NB).reshape(128, 1),
    ], axis=1).astype(f32)
    import ml_dtypes
    wbf_np = np.ascontiguousarray(
        np.concatenate([wr_t, wz_t, wn_t], axis=1).astype(ml_dtypes.bfloat16))
    shared = {"consts": np.ascontiguousarray(consts_np), "wbf": wbf_np}

    in_maps = []
    for b in range(NCORES):
        m = dict(shared)
        m["resT"] = np.ascontiguousarray(residual[b].T)
        in_maps.append(m)

    if _NC_CACHE is None:
        _NC_CACHE = _build_nc()
    nc = _NC_CACHE

    res = run_bass_kernel_spmd(nc, in_maps, core_ids=list(range(NCORES)))
    if res.exec_time_ns is not None:
        print(f"HW exec time: {res.exec_time_ns} ns")

    out = np.zeros((B, S, H), f32)
    for b in range(NCORES):
        hb = res.results[b]["out"]                     # [128, L] blocked
        out[b] = hb.reshape(NB, H, L).transpose(0, 2, 1).reshape(S, H)
    return out


# revision 31
# speedup vs baseline: 1.2301x; 1.2301x over previous
"""Trainium2 Bass kernel for nn_AffectChannel (compress + GELU + 16-dim GRU scan).

Strategy (8 NeuronCores, data-parallel over batch, one batch element per core):
  Phase 1 (memory-bound): compressed = gelu(residual @ compress_w.T + b)
    - residual shard is pre-transposed on host -> fully coalesced DMA, fp32
      matmuls contract d on partitions, accumulate in PSUM.
  Phase 2: x_gates = compressed @ w_ih.T + biases, written in a "blocked"
    layout: partitions = 16 hidden lanes x 8 time-blocks (512 steps each).
  Phase 3: the sequential GRU scan is computed by Picard iteration: gates are
    evaluated from the previous trajectory estimate (fully parallel, 128-lane
    ops), then the diagonal blend recurrence h_t = z_t h_{t-1} + (1-z_t) n_t
    is solved EXACTLY with the DVE tensor_tensor_scan instruction (per-block
    prefix scans + an 8-block boundary chain via tiny PE gather/scatter
    matmuls).  ~24 sweeps converge to fp32 accuracy (contraction ~0.5/sweep).
"""
import json
import os

import numpy as np

B, S, D, C, H = 8, 4096, 2048, 64, 16
NB = 8           # time blocks
L = S // NB      # block length = 512
NCORES = 8
N_SWEEP = int(os.environ.get("AFFECT_N_SWEEP", "23"))
N_BF = int(os.environ.get("AFFECT_N_BF", "13"))


# --- walrus workaround: split multi-wait instructions ----------------------
def _split_multiwaits(d):
    n = 0
    uid = [0]
    for f in d.get("functions", []):
        for blk in f.get("blocks", []):
            out = []
            for ins in blk.get("instructions", []):
                si = ins.get("sync_info")
                waits = (si or {}).get("on_wait") or []
                if len(waits) > 1:
                    n += 1
                    for w in waits[:-1]:
                        uid[0] += 1
                        out.append({
                            "opcode": "EventSemaphore",
                            "name": f"{ins['name']}_wsplit{uid[0]}",
                            "engine": ins["engine"],
                            "ins": [], "outs": [],
                            "debug": ins.get("debug"),
                            "sync_info": {"on_wait": [w], "on_update": []},
                        })
                    si["on_wait"] = [waits[-1]]
                out.append(ins)
            blk["instructions"] = out
    return n


def _fix_bir_json(bir_json):
    if isinstance(bir_json, str):
        bir_json = bir_json.encode()
    d = json.loads(bir_json)
    if _split_multiwaits(d) == 0:
        return bir_json
    return json.dumps(d).encode()


_PATCHED = False


def _install_bir_fix():
    global _PATCHED
    if _PATCHED:
        return
    _PATCHED = True
    import concourse.bass_utils as bu
    import concourse.bass2jax as b2j

    orig = bu.compile_bir_kernel

    def patched(bir_json, tmpdir, neff_name="file.neff"):
        return orig(_fix_bir_json(bir_json), tmpdir, neff_name=neff_name)

    bu.compile_bir_kernel = patched
    b2j.compile_bir_kernel = patched


# --- kernel build ----------------------------------------------------------
def _build_nc():
    import concourse.bass as bass
    import concourse.mybir as mybir
    from concourse.tile import TileContext
    from concourse.tile import add_dep_helper

    F32 = mybir.dt.float32
    BF16 = mybir.dt.bfloat16
    AF = mybir.ActivationFunctionType
    OP = mybir.AluOpType
    AX = mybir.AxisListType

    nc = bass.Bass("TRN2", target_bir_lowering=False)

    resT = nc.dram_tensor("resT", [D, S], F32, kind="ExternalInput")
    # all constants packed into one tensor: [cw(1024) wr wz wn(128*3) wih2*(32*3)
    #  gmat(16) smat_pad(128) maskb selm(8+8) cols(1*5)] = 1669 cols
    consts = nc.dram_tensor("consts", [128, 1669], F32, kind="ExternalInput")
    wbf = nc.dram_tensor("wbf", [128, 384], mybir.dt.bfloat16, kind="ExternalInput")
    out = nc.dram_tensor("out", [128, L], F32, kind="ExternalOutput")

    NDC = D // 128  # 16 d-chunks

    with TileContext(nc) as tc:
        with tc.tile_pool(name="const", bufs=1) as cst, \
             tc.tile_pool(name="persist", bufs=1) as per:
            call = cst.tile([128, 1669], F32, tag="call")
            nc.sync.dma_start(call[:], consts.ap())
            cw_sb = call[:, 0:1024]
            wr_sb = call[:, 1024:1152]
            wz_sb = call[:, 1152:1280]
            wn_sb = call[:, 1280:1408]
            wih2r_sb = call[:, 1408:1440]
            wih2z_sb = call[:, 1440:1472]
            wih2n_sb = call[:, 1472:1504]
            g_sb = call[:, 1504:1520]
            s_sb = call[0:16, 1520:1648]
            mb_sb = call[:, 1648:1656]
            sel_sb = call[:, 1656:1664]
            cb2_sb = call[:, 1664:1665]
            brc_sb = call[:, 1665:1666]
            bzc_sb = call[:, 1666:1667]
            bnc_sb = call[:, 1667:1668]
            bhn_sb = call[:, 1668:1669]
            wbf_sb = cst.tile([128, 384], BF16, tag="wbf")
            nc.sync.dma_start(wbf_sb[:], wbf.ap())
            wrb_sb = wbf_sb[:, 0:128]
            wzb_sb = wbf_sb[:, 128:256]
            wnb_sb = wbf_sb[:, 256:384]
            ones_sb = cst.tile([128, L], F32, tag="ones")
            nc.vector.memset(ones_sb, 1.0)

            # chunk-pair layout: rows 0-63 = even s-chunks, 64-127 = odd
            comp2 = per.tile([128, S // 2], F32, tag="comp2")
            xrz_sb = per.tile([128, 2 * L], F32, tag="xrz")
            xn_sb = per.tile([128, L], F32, tag="xn")

            # ---- Phase 1: compress matmul + gelu -------------------------
            # s-chunk pairs run CONCURRENTLY on the PE via column tiling:
            # even chunk -> array col strips 0-63, odd chunk -> 64-127.
            # The [128, L] psum pair tile is already in comp2's layout.
            with tc.tile_pool(name="resp", bufs=4) as resp, \
                 tc.tile_pool(name="cpsum", bufs=1, space="PSUM") as cpsum:
                ctiles = [cpsum.tile([128, L], F32, tag=f"c{p}", name=f"c{p}") for p in range(4)]
                for dc in range(NDC):
                    rt = resp.tile([128, S], F32, tag="res")
                    # split each 2MiB row-chunk across both HWDGE rings
                    nc.sync.dma_start(
                        rt[:, 0:S // 2],
                        resT.ap()[dc * 128:(dc + 1) * 128, 0:S // 2])
                    nc.scalar.dma_start(
                        rt[:, S // 2:S],
                        resT.ap()[dc * 128:(dc + 1) * 128, S // 2:S])
                    for p in range(4):
                        nc.tensor.matmul(
                            ctiles[p][0:64, :],
                            cw_sb[:, dc * C:(dc + 1) * C],
                            rt[:, (2 * p) * L:(2 * p + 1) * L],
                            start=(dc == 0), stop=(dc == NDC - 1),
                            tile_position=(0, 0),
                        )
                        nc.tensor.matmul(
                            ctiles[p][64:128, :],
                            cw_sb[:, dc * C:(dc + 1) * C],
                            rt[:, (2 * p + 1) * L:(2 * p + 2) * L],
                            start=(dc == 0), stop=(dc == NDC - 1),
                            tile_position=(0, 64),
                        )
                for p in range(4):
                    nc.scalar.activation(
                        comp2[:, L * p:L * (p + 1)],
                        ctiles[p][:], AF.Gelu, bias=cb2_sb,
                    )

            # ---- Phase 2: x-gates directly into blocked layout -----------
            # lhsT = blockdiag([wihT_g, wihT_g]) over the chunk-pair rows of
            # comp2 -> out [32, L] at partition 32j = blocks 2j (rows 0-15)
            # and 2j+1 (rows 16-31).
            with tc.tile_pool(name="xpsum", bufs=1, space="PSUM") as xpsum:
                for g, (wt2, dst, bias) in enumerate([
                    (wih2r_sb, xrz_sb[:, 0:L], brc_sb),
                    (wih2z_sb, xrz_sb[:, L:2 * L], bzc_sb),
                    (wih2n_sb, xn_sb[:], bnc_sb),
                ]):
                    ps = xpsum.tile([128, L], F32, tag=f"xg{g}", name=f"xg{g}")
                    for j in range(4):
                        nc.tensor.matmul(
                            ps[32 * j:32 * j + 32, :], wt2[:],
                            comp2[:, j * L:(j + 1) * L],
                            start=True, stop=True,
                            tile_position=(0, 32 * j),
                        )
                    nc.scalar.activation(
                        dst, ps[:], AF.Identity, bias=bias[:, 0:1],
                    )

            # ---- Phase 3: Picard sweeps ----------------------------------
            # hs tiles hold the SHIFTED trajectory: hs[:, 0] = boundary
            # column (end of previous block = h_{t-1} for the block start),
            # hs[:, 1:L] = h[:, 0:L-1].  Gate matmuls then read hs directly.
            with tc.tile_pool(name="spsum", bufs=1, space="PSUM") as sps, \
                 tc.tile_pool(name="swp", bufs=1) as swp:
                rzps = sps.tile([128, 2 * L], F32, tag="rzps")
                wps = sps.tile([128, L], F32, tag="wps")
                ptps = sps.tile([16, NB], F32, tag="ptps")
                otps = sps.tile([16, NB], F32, tag="otps")
                ebps = sps.tile([128, NB], F32, tag="ebps")

                hs0 = swp.tile([128, L], F32, tag="hs0")
                hs1 = swp.tile([128, L], F32, tag="hs1")
                h_out = swp.tile([128, L], F32, tag="hout")
                s_t = swp.tile([128, 2 * L], F32, tag="st")
                rz = swp.tile([128, 2 * L], F32, tag="rz")
                u_t = swp.tile([128, L], F32, tag="ut")
                v_t = swp.tile([128, L], F32, tag="vt")
                n_t = swp.tile([128, L], F32, tag="nt")
                c_t = swp.tile([128, L], F32, tag="ct")
                P_t = swp.tile([128, L], F32, tag="Pt")
                O_t = swp.tile([128, L], F32, tag="Ot")
                pm = swp.tile([128, NB], F32, tag="pm")
                om = swp.tile([128, NB], F32, tag="om")
                ot_sb = swp.tile([16, NB], F32, tag="otsb")
                hb_sb = swp.tile([16, NB], F32, tag="hbsb")
                eb_sb = swp.tile([128, NB], F32, tag="ebsb")

                hsb0 = swp.tile([128, L], BF16, tag="hsb0")
                hsb1 = swp.tile([128, L], BF16, tag="hsb1")
                hbq0 = swp.tile([128, 1], F32, tag="hbq0")
                hbq1 = swp.tile([128, 1], F32, tag="hbq1")

                nc.vector.memset(hsb0[:], 0.0)
                nc.vector.memset(hs0[:], 0.0)
                nc.vector.memset(hbq0[:], 0.0)
                hbq_cur, hbq_nxt = hbq0, hbq1
                hb_cur, hb_nxt = hsb0, hsb1
                h_cur, h_nxt = hs0, hs1
                for k in range(N_SWEEP):
                    # first N_BF sweeps evaluate the gate matmuls in bf16
                    # (1 PE pass instead of fp32's 2); PSUM accumulation and
                    # everything downstream stays fp32.  The fp32 tail
                    # re-converges to the exact fp32 fixpoint.
                    bf_gates = k < N_BF
                    bf_write = k < N_BF - 1
                    if bf_gates:
                        gate_list = ((wrb_sb, rzps[:, 0:L]),
                                     (wzb_sb, rzps[:, L:2 * L]),
                                     (wnb_sb, wps[:]))
                        rhs = hb_cur
                    else:
                        gate_list = ((wr_sb, rzps[:, 0:L]),
                                     (wz_sb, rzps[:, L:2 * L]),
                                     (wn_sb, wps[:]))
                        rhs = h_cur
                    for wt, ps in gate_list:
                        nc.tensor.matmul(
                            ps[:], wt, rhs[:], start=True, stop=True)
                    nc.vector.tensor_tensor(s_t[:], rzps[:], xrz_sb[:], OP.add)
                    nc.scalar.activation(rz[:], s_t[:], AF.Sigmoid)
                    nc.vector.scalar_tensor_tensor(
                        u_t[:], wps[:], bhn_sb, rz[:, 0:L], OP.add, OP.mult)
                    v_bi = nc.vector.tensor_tensor(v_t[:], u_t[:], xn_sb[:], OP.add)
                    nc.scalar.activation(n_t[:], v_t[:], AF.Tanh)
                    # P-scan is off the critical chain; schedule it on DVE
                    # after v so it hides behind the tanh instead of
                    # delaying the u->v->tanh chain.
                    p_bi = nc.vector.tensor_tensor_scan(
                        P_t[:], rz[:, L:2 * L], ones_sb, 1.0, OP.mult, OP.mult)
                    add_dep_helper(v_bi.ins, p_bi.ins, sync=False,
                                   reason="P-scan after v on DVE")
                    nc.vector.scalar_tensor_tensor(
                        c_t[:], rz[:, L:2 * L], 1.0, n_t[:], OP.subtract, OP.mult)
                    nc.vector.tensor_tensor_scan(
                        O_t[:], rz[:, L:2 * L], c_t[:], 0.0, OP.mult, OP.subtract)
                    # combine with the LAGGED boundary column (computed from
                    # the previous sweep's P/O) - the boundary only shifts
                    # which iterate we are on, not the fixpoint, and costs
                    # nothing in convergence (verified numerically).
                    if bf_write:
                        cmb_bi = nc.vector.scalar_tensor_tensor(
                            hb_nxt[:, 1:L], P_t[:, 0:L - 1], hbq_cur[:, 0:1],
                            O_t[:, 0:L - 1], OP.mult, OP.add)
                        nc.vector.tensor_copy(hb_nxt[:, 0:1], hbq_cur[:])
                        hb_cur, hb_nxt = hb_nxt, hb_cur
                    else:
                        cmb_bi = nc.vector.scalar_tensor_tensor(
                            h_nxt[:, 1:L], P_t[:, 0:L - 1], hbq_cur[:, 0:1],
                            O_t[:, 0:L - 1], OP.mult, OP.add)
                        nc.vector.tensor_copy(h_nxt[:, 0:1], hbq_cur[:])
                        if k == N_SWEEP - 1:
                            nc.vector.scalar_tensor_tensor(
                                h_out[:], P_t[:], hbq_cur[:, 0:1], O_t[:],
                                OP.mult, OP.add)
                        h_cur, h_nxt = h_nxt, h_cur
                    # boundary chain for the NEXT sweep - off the critical
                    # path; overlaps the next sweep's gate/sigmoid stretch.
                    pm_bi = nc.vector.tensor_scalar(
                        pm[:], mb_sb, P_t[:, L - 1:L], None, OP.mult)
                    add_dep_helper(cmb_bi.ins, pm_bi.ins, sync=False,
                                   reason="boundary after combine on DVE")
                    nc.vector.tensor_scalar(
                        om[:], mb_sb, O_t[:, L - 1:L], None, OP.mult)
                    nc.tensor.matmul(ptps[:], g_sb, pm[:], start=True, stop=True)
                    nc.tensor.matmul(otps[:], g_sb, om[:], start=True, stop=True)
                    nc.vector.tensor_copy(ot_sb[:], otps[:])
                    nc.vector.tensor_tensor_scan(
                        hb_sb[:], ptps[:], ot_sb[:], 0.0, OP.mult, OP.add)
                    nc.tensor.matmul(ebps[:], s_sb, hb_sb[:], start=True, stop=True)
                    nc.vector.tensor_tensor(eb_sb[:], ebps[:], sel_sb, OP.mult)
                    nc.vector.tensor_reduce(hbq_nxt[:], eb_sb[:], AX.X, OP.add)
                    hbq_cur, hbq_nxt = hbq_nxt, hbq_cur

                nc.sync.dma_start(out.ap(), h_out[:])

    return nc


_NC_CACHE = None


def kernel(residual, compress_w, compress_b, w_ih, w_hh, b_ih, b_hh):
    global _NC_CACHE
    _install_bir_fix()
    from concourse.bass_utils import run_bass_kernel_spmd

    f32 = np.float32
    residual = np.ascontiguousarray(residual, dtype=f32)
    compress_w = np.asarray(compress_w, dtype=f32)
    compress_b = np.asarray(compress_b, dtype=f32)
    w_ih = np.asarray(w_ih, dtype=f32)
    w_hh = np.asarray(w_hh, dtype=f32)
    b_ih = np.asarray(b_ih, dtype=f32)
    b_hh = np.asarray(b_hh, dtype=f32)

    # host-side shared weight prep (layout only)
    cwT = np.ascontiguousarray(compress_w.T)                      # [D, C]
    cw_tiles = np.ascontiguousarray(
        cwT.reshape(D // 128, 128, C).transpose(1, 0, 2).reshape(128, -1))
    wihT = np.ascontiguousarray(w_ih.T)                           # [C, 3H]

    def wih2(g):   # [128, 32] blockdiag over chunk-pair rows
        m = np.zeros((128, 32), f32)
        m[0:C, 0:16] = wihT[:, 16 * g:16 * g + 16]
        m[C:2 * C, 16:32] = wihT[:, 16 * g:16 * g + 16]
        return m

    def blockdiag_tiled(w):   # w: [H, H] -> [128, 128] (8 diagonal blocks)
        return np.ascontiguousarray(np.kron(np.eye(NB, dtype=f32), w.T.astype(f32)))

    wr_t = blockdiag_tiled(w_hh[:H])
    wz_t = blockdiag_tiled(w_hh[H:2 * H])
    wn_t = blockdiag_tiled(w_hh[2 * H:])

    g_np = np.zeros((128, 16), f32)
    for kk in range(128):
        g_np[kk, kk % 16] = 1.0
    s_np = np.ascontiguousarray(g_np.T)
    maskb_np = np.zeros((128, NB), f32)
    sel_np = np.zeros((128, NB), f32)
    for kk in range(128):
        maskb_np[kk, kk // 16] = 1.0
        if kk // 16 >= 1:
            sel_np[kk, kk // 16 - 1] = 1.0

    smat_pad = np.zeros((128, 128), f32)
    smat_pad[0:16, :] = s_np
    consts_np = np.concatenate([
        cw_tiles, wr_t, wz_t, wn_t, wih2(0), wih2(1), wih2(2), g_np, smat_pad,
        maskb_np, sel_np,
        np.tile(compress_b, 2).reshape(128, 1),
        np.tile(b_ih[:H] + b_hh[:H], NB).reshape(128, 1),
        np.tile(b_ih[H:2 * H] + b_hh[H:2 * H], NB).reshape(128, 1),
        np.tile(b_ih[2 * H:], NB).reshape(128, 1),
        np.tile(b_hh[2 * H:], NB).reshape(128, 1),
    ], axis=1).astype(f32)
    import ml_dtypes
    wbf_np = np.ascontiguousarray(
        np.concatenate([wr_t, wz_t, wn_t], axis=1).astype(ml_dtypes.bfloat16))
    shared = {"consts": np.ascontiguousarray(consts_np), "wbf": wbf_np}

    in_maps = []
    for b in range(NCORES):
        m = dict(shared)
        m["resT"] = np.ascontiguousarray(residual[b].T)
        in_maps.append(m)

    if _NC_CACHE is None:
        _NC_CACHE = _build_nc()
    nc = _NC_CACHE

    res = run_bass_kernel_spmd(nc, in_maps, core_ids=list(range(NCORES)))
    if res.exec_time_ns is not None:
        print(f"HW exec time: {res.exec_time_ns} ns")

    out = np.zeros((B, S, H), f32)
    for b in range(NCORES):
        hb = res.results[b]["out"]                     # [128, L] blocked
        out[b] = hb.reshape(NB, H, L).transpose(0, 2, 1).reshape(S, H)
    return out


# revision 32
# speedup vs baseline: 1.3408x; 1.0900x over previous
"""Trainium2 Bass kernel for nn_AffectChannel (compress + GELU + 16-dim GRU scan).

Strategy (8 NeuronCores, data-parallel over batch, one batch element per core):
  Phase 1 (memory-bound): compressed = gelu(residual @ compress_w.T + b)
    - residual shard is pre-transposed on host -> fully coalesced DMA, fp32
      matmuls contract d on partitions, accumulate in PSUM.
  Phase 2: x_gates = compressed @ w_ih.T + biases, written in a "blocked"
    layout: partitions = 16 hidden lanes x 8 time-blocks (512 steps each).
  Phase 3: the sequential GRU scan is computed by Picard iteration: gates are
    evaluated from the previous trajectory estimate (fully parallel, 128-lane
    ops), then the diagonal blend recurrence h_t = z_t h_{t-1} + (1-z_t) n_t
    is solved EXACTLY with the DVE tensor_tensor_scan instruction (per-block
    prefix scans + an 8-block boundary chain via tiny PE gather/scatter
    matmuls).  ~24 sweeps converge to fp32 accuracy (contraction ~0.5/sweep).
"""
import json
import os

import numpy as np

B, S, D, C, H = 8, 4096, 2048, 64, 16
NB = 8           # time blocks
L = S // NB      # block length = 512
NCORES = 8
N_SWEEP = int(os.environ.get("AFFECT_N_SWEEP", "22"))
N_BF = int(os.environ.get("AFFECT_N_BF", "13"))


# --- walrus workaround: split multi-wait instructions ----------------------
def _split_multiwaits(d):
    n = 0
    uid = [0]
    for f in d.get("functions", []):
        for blk in f.get("blocks", []):
            out = []
            for ins in blk.get("instructions", []):
                si = ins.get("sync_info")
                waits = (si or {}).get("on_wait") or []
                if len(waits) > 1:
                    n += 1
                    for w in waits[:-1]:
                        uid[0] += 1
                        out.append({
                            "opcode": "EventSemaphore",
                            "name": f"{ins['name']}_wsplit{uid[0]}",
                            "engine": ins["engine"],
                            "ins": [], "outs": [],
                            "debug": ins.get("debug"),
                            "sync_info": {"on_wait": [w], "on_update": []},
                        })
                    si["on_wait"] = [waits[-1]]
                out.append(ins)
            blk["instructions"] = out
    return n


def _fix_bir_json(bir_json):
    if isinstance(bir_json, str):
        bir_json = bir_json.encode()
    d = json.loads(bir_json)
    if _split_multiwaits(d) == 0:
        return bir_json
    return json.dumps(d).encode()


_PATCHED = False


def _install_bir_fix():
    global _PATCHED
    if _PATCHED:
        return
    _PATCHED = True
    import concourse.bass_utils as bu
    import concourse.bass2jax as b2j

    orig = bu.compile_bir_kernel

    def patched(bir_json, tmpdir, neff_name="file.neff"):
        return orig(_fix_bir_json(bir_json), tmpdir, neff_name=neff_name)

    bu.compile_bir_kernel = patched
    b2j.compile_bir_kernel = patched


# --- kernel build ----------------------------------------------------------
def _build_nc():
    import concourse.bass as bass
    import concourse.mybir as mybir
    from concourse.tile import TileContext
    from concourse.tile import add_dep_helper

    F32 = mybir.dt.float32
    BF16 = mybir.dt.bfloat16
    AF = mybir.ActivationFunctionType
    OP = mybir.AluOpType
    AX = mybir.AxisListType

    nc = bass.Bass("TRN2", target_bir_lowering=False)

    resT = nc.dram_tensor("resT", [D, S], F32, kind="ExternalInput")
    # all constants packed into one tensor: [cw(1024) wr wz wn(128*3) wih2*(32*3)
    #  gmat(16) smat_pad(128) maskb selm(8+8) cols(1*5)] = 1669 cols
    consts = nc.dram_tensor("consts", [128, 1669], F32, kind="ExternalInput")
    wbf = nc.dram_tensor("wbf", [128, 384], mybir.dt.bfloat16, kind="ExternalInput")
    out = nc.dram_tensor("out", [128, L], F32, kind="ExternalOutput")

    NDC = D // 128  # 16 d-chunks

    with TileContext(nc) as tc:
        with tc.tile_pool(name="const", bufs=1) as cst, \
             tc.tile_pool(name="persist", bufs=1) as per:
            call = cst.tile([128, 1669], F32, tag="call")
            nc.sync.dma_start(call[:], consts.ap())
            cw_sb = call[:, 0:1024]
            wr_sb = call[:, 1024:1152]
            wz_sb = call[:, 1152:1280]
            wn_sb = call[:, 1280:1408]
            wih2r_sb = call[:, 1408:1440]
            wih2z_sb = call[:, 1440:1472]
            wih2n_sb = call[:, 1472:1504]
            g_sb = call[:, 1504:1520]
            s_sb = call[0:16, 1520:1648]
            mb_sb = call[:, 1648:1656]
            sel_sb = call[:, 1656:1664]
            cb2_sb = call[:, 1664:1665]
            brc_sb = call[:, 1665:1666]
            bzc_sb = call[:, 1666:1667]
            bnc_sb = call[:, 1667:1668]
            bhn_sb = call[:, 1668:1669]
            wbf_sb = cst.tile([128, 384], BF16, tag="wbf")
            nc.sync.dma_start(wbf_sb[:], wbf.ap())
            wrb_sb = wbf_sb[:, 0:128]
            wzb_sb = wbf_sb[:, 128:256]
            wnb_sb = wbf_sb[:, 256:384]
            ones_sb = cst.tile([128, L], F32, tag="ones")
            nc.vector.memset(ones_sb, 1.0)

            # chunk-pair layout: rows 0-63 = even s-chunks, 64-127 = odd
            comp2 = per.tile([128, S // 2], F32, tag="comp2")
            xrz_sb = per.tile([128, 2 * L], F32, tag="xrz")
            xn_sb = per.tile([128, L], F32, tag="xn")

            # ---- Phase 1: compress matmul + gelu -------------------------
            # s-chunk pairs run CONCURRENTLY on the PE via column tiling:
            # even chunk -> array col strips 0-63, odd chunk -> 64-127.
            # The [128, L] psum pair tile is already in comp2's layout.
            with tc.tile_pool(name="resp", bufs=4) as resp, \
                 tc.tile_pool(name="cpsum", bufs=1, space="PSUM") as cpsum:
                ctiles = [cpsum.tile([128, L], F32, tag=f"c{p}", name=f"c{p}") for p in range(4)]
                for dc in range(NDC):
                    rt = resp.tile([128, S], F32, tag="res")
                    # split each 2MiB row-chunk across both HWDGE rings
                    nc.sync.dma_start(
                        rt[:, 0:S // 2],
                        resT.ap()[dc * 128:(dc + 1) * 128, 0:S // 2])
                    nc.scalar.dma_start(
                        rt[:, S // 2:S],
                        resT.ap()[dc * 128:(dc + 1) * 128, S // 2:S])
                    for p in range(4):
                        nc.tensor.matmul(
                            ctiles[p][0:64, :],
                            cw_sb[:, dc * C:(dc + 1) * C],
                            rt[:, (2 * p) * L:(2 * p + 1) * L],
                            start=(dc == 0), stop=(dc == NDC - 1),
                            tile_position=(0, 0),
                        )
                        nc.tensor.matmul(
                            ctiles[p][64:128, :],
                            cw_sb[:, dc * C:(dc + 1) * C],
                            rt[:, (2 * p + 1) * L:(2 * p + 2) * L],
                            start=(dc == 0), stop=(dc == NDC - 1),
                            tile_position=(0, 64),
                        )
                for p in range(4):
                    nc.scalar.activation(
                        comp2[:, L * p:L * (p + 1)],
                        ctiles[p][:], AF.Gelu, bias=cb2_sb,
                    )

            # ---- Phase 2: x-gates directly into blocked layout -----------
            # lhsT = blockdiag([wihT_g, wihT_g]) over the chunk-pair rows of
            # comp2 -> out [32, L] at partition 32j = blocks 2j (rows 0-15)
            # and 2j+1 (rows 16-31).
            with tc.tile_pool(name="xpsum", bufs=1, space="PSUM") as xpsum:
                for g, (wt2, dst, bias) in enumerate([
                    (wih2r_sb, xrz_sb[:, 0:L], brc_sb),
                    (wih2z_sb, xrz_sb[:, L:2 * L], bzc_sb),
                    (wih2n_sb, xn_sb[:], bnc_sb),
                ]):
                    ps = xpsum.tile([128, L], F32, tag=f"xg{g}", name=f"xg{g}")
                    for j in range(4):
                        nc.tensor.matmul(
                            ps[32 * j:32 * j + 32, :], wt2[:],
                            comp2[:, j * L:(j + 1) * L],
                            start=True, stop=True,
                            tile_position=(0, 32 * j),
                        )
                    nc.scalar.activation(
                        dst, ps[:], AF.Identity, bias=bias[:, 0:1],
                    )

            # ---- Phase 3: Picard sweeps ----------------------------------
            # hs tiles hold the SHIFTED trajectory: hs[:, 0] = boundary
            # column (end of previous block = h_{t-1} for the block start),
            # hs[:, 1:L] = h[:, 0:L-1].  Gate matmuls then read hs directly.
            with tc.tile_pool(name="spsum", bufs=1, space="PSUM") as sps, \
                 tc.tile_pool(name="swp", bufs=1) as swp:
                rzps = sps.tile([128, 2 * L], F32, tag="rzps")
                wps = sps.tile([128, L], F32, tag="wps")
                ptps = sps.tile([16, NB], F32, tag="ptps")
                otps = sps.tile([16, NB], F32, tag="otps")
                ebps = sps.tile([128, NB], F32, tag="ebps")

                hs0 = swp.tile([128, L], F32, tag="hs0")
                hs1 = swp.tile([128, L], F32, tag="hs1")
                h_out = swp.tile([128, L], F32, tag="hout")
                s_t = swp.tile([128, 2 * L], F32, tag="st")
                rz = swp.tile([128, 2 * L], F32, tag="rz")
                u_t = swp.tile([128, L], F32, tag="ut")
                v_t = swp.tile([128, L], F32, tag="vt")
                n_t = swp.tile([128, L], F32, tag="nt")
                c_t = swp.tile([128, L], F32, tag="ct")
                P_t = swp.tile([128, L], F32, tag="Pt")
                O_t = swp.tile([128, L], F32, tag="Ot")
                pm = swp.tile([128, NB], F32, tag="pm")
                om = swp.tile([128, NB], F32, tag="om")
                ot_sb = swp.tile([16, NB], F32, tag="otsb")
                hb_sb = swp.tile([16, NB], F32, tag="hbsb")
                eb_sb = swp.tile([128, NB], F32, tag="ebsb")

                hsb0 = swp.tile([128, L], BF16, tag="hsb0")
                hsb1 = swp.tile([128, L], BF16, tag="hsb1")
                hbq0 = swp.tile([128, 1], F32, tag="hbq0")
                hbq1 = swp.tile([128, 1], F32, tag="hbq1")

                nc.vector.memset(hsb0[:], 0.0)
                nc.vector.memset(hs0[:], 0.0)
                nc.vector.memset(hbq0[:], 0.0)
                hbq_cur, hbq_nxt = hbq0, hbq1
                hb_cur, hb_nxt = hsb0, hsb1
                h_cur, h_nxt = hs0, hs1
                for k in range(N_SWEEP):
                    # first N_BF sweeps evaluate the gate matmuls in bf16
                    # (1 PE pass instead of fp32's 2); PSUM accumulation and
                    # everything downstream stays fp32.  The fp32 tail
                    # re-converges to the exact fp32 fixpoint.
                    bf_gates = k < N_BF
                    bf_write = k < N_BF - 1
                    if bf_gates:
                        gate_list = ((wrb_sb, rzps[:, 0:L]),
                                     (wzb_sb, rzps[:, L:2 * L]),
                                     (wnb_sb, wps[:]))
                        rhs = hb_cur
                    else:
                        gate_list = ((wr_sb, rzps[:, 0:L]),
                                     (wz_sb, rzps[:, L:2 * L]),
                                     (wn_sb, wps[:]))
                        rhs = h_cur
                    if k == 0:
                        # h = 0: gates reduce to sigma(x); u = bhn * r
                        nc.scalar.activation(rz[:], xrz_sb[:], AF.Sigmoid)
                        nc.vector.tensor_scalar(
                            u_t[:], rz[:, 0:L], bhn_sb[:, 0:1], None, OP.mult)
                    else:
                        for wt, ps in gate_list:
                            nc.tensor.matmul(
                                ps[:], wt, rhs[:], start=True, stop=True)
                        nc.vector.tensor_tensor(s_t[:], rzps[:], xrz_sb[:], OP.add)
                        nc.scalar.activation(rz[:], s_t[:], AF.Sigmoid)
                        nc.vector.scalar_tensor_tensor(
                            u_t[:], wps[:], bhn_sb, rz[:, 0:L], OP.add, OP.mult)
                    v_bi = nc.vector.tensor_tensor(v_t[:], u_t[:], xn_sb[:], OP.add)
                    nc.scalar.activation(n_t[:], v_t[:], AF.Tanh)
                    # P-scan is off the critical chain; schedule it on DVE
                    # after v so it hides behind the tanh instead of
                    # delaying the u->v->tanh chain.
                    p_bi = nc.vector.tensor_tensor_scan(
                        P_t[:], rz[:, L:2 * L], ones_sb, 1.0, OP.mult, OP.mult)
                    add_dep_helper(v_bi.ins, p_bi.ins, sync=False,
                                   reason="P-scan after v on DVE")
                    nc.vector.scalar_tensor_tensor(
                        c_t[:], rz[:, L:2 * L], 1.0, n_t[:], OP.subtract, OP.mult)
                    nc.vector.tensor_tensor_scan(
                        O_t[:], rz[:, L:2 * L], c_t[:], 0.0, OP.mult, OP.subtract)
                    # combine with the LAGGED boundary column (computed from
                    # the previous sweep's P/O) - the boundary only shifts
                    # which iterate we are on, not the fixpoint, and costs
                    # nothing in convergence (verified numerically).
                    if bf_write:
                        cmb_bi = nc.vector.scalar_tensor_tensor(
                            hb_nxt[:, 1:L], P_t[:, 0:L - 1], hbq_cur[:, 0:1],
                            O_t[:, 0:L - 1], OP.mult, OP.add)
                        nc.vector.tensor_copy(hb_nxt[:, 0:1], hbq_cur[:])
                        hb_cur, hb_nxt = hb_nxt, hb_cur
                    else:
                        cmb_bi = nc.vector.scalar_tensor_tensor(
                            h_nxt[:, 1:L], P_t[:, 0:L - 1], hbq_cur[:, 0:1],
                            O_t[:, 0:L - 1], OP.mult, OP.add)
                        nc.vector.tensor_copy(h_nxt[:, 0:1], hbq_cur[:])
                        if k == N_SWEEP - 1:
                            nc.vector.scalar_tensor_tensor(
                                h_out[:], P_t[:], hbq_cur[:, 0:1], O_t[:],
                                OP.mult, OP.add)
                        h_cur, h_nxt = h_nxt, h_cur
                    # boundary chain for the NEXT sweep - off the critical
                    # path; overlaps the next sweep's gate/sigmoid stretch.
                    pm_bi = nc.vector.tensor_scalar(
                        pm[:], mb_sb, P_t[:, L - 1:L], None, OP.mult)
                    add_dep_helper(cmb_bi.ins, pm_bi.ins, sync=False,
                                   reason="boundary after combine on DVE")
                    nc.vector.tensor_scalar(
                        om[:], mb_sb, O_t[:, L - 1:L], None, OP.mult)
                    nc.tensor.matmul(ptps[:], g_sb, pm[:], start=True, stop=True)
                    nc.tensor.matmul(otps[:], g_sb, om[:], start=True, stop=True)
                    nc.vector.tensor_copy(ot_sb[:], otps[:])
                    nc.vector.tensor_tensor_scan(
                        hb_sb[:], ptps[:], ot_sb[:], 0.0, OP.mult, OP.add)
                    nc.tensor.matmul(ebps[:], s_sb, hb_sb[:], start=True, stop=True)
                    nc.vector.tensor_tensor(eb_sb[:], ebps[:], sel_sb, OP.mult)
                    nc.vector.tensor_reduce(hbq_nxt[:], eb_sb[:], AX.X, OP.add)
                    hbq_cur, hbq_nxt = hbq_nxt, hbq_cur

                nc.sync.dma_start(out.ap(), h_out[:])

    return nc


_NC_CACHE = None


def kernel(residual, compress_w, compress_b, w_ih, w_hh, b_ih, b_hh):
    global _NC_CACHE
    _install_bir_fix()
    from concourse.bass_utils import run_bass_kernel_spmd

    f32 = np.float32
    residual = np.ascontiguousarray(residual, dtype=f32)
    compress_w = np.asarray(compress_w, dtype=f32)
    compress_b = np.asarray(compress_b, dtype=f32)
    w_ih = np.asarray(w_ih, dtype=f32)
    w_hh = np.asarray(w_hh, dtype=f32)
    b_ih = np.asarray(b_ih, dtype=f32)
    b_hh = np.asarray(b_hh, dtype=f32)

    # host-side shared weight prep (layout only)
    cwT = np.ascontiguousarray(compress_w.T)                      # [D, C]
    cw_tiles = np.ascontiguousarray(
        cwT.reshape(D // 128, 128, C).transpose(1, 0, 2).reshape(128, -1))
    wihT = np.ascontiguousarray(w_ih.T)                           # [C, 3H]

    def wih2(g):   # [128, 32] blockdiag over chunk-pair rows
        m = np.zeros((128, 32), f32)
        m[0:C, 0:16] = wihT[:, 16 * g:16 * g + 16]
        m[C:2 * C, 16:32] = wihT[:, 16 * g:16 * g + 16]
        return m

    def blockdiag_tiled(w):   # w: [H, H] -> [128, 128] (8 diagonal blocks)
        return np.ascontiguousarray(np.kron(np.eye(NB, dtype=f32), w.T.astype(f32)))

    wr_t = blockdiag_tiled(w_hh[:H])
    wz_t = blockdiag_tiled(w_hh[H:2 * H])
    wn_t = blockdiag_tiled(w_hh[2 * H:])

    g_np = np.zeros((128, 16), f32)
    for kk in range(128):
        g_np[kk, kk % 16] = 1.0
    s_np = np.ascontiguousarray(g_np.T)
    maskb_np = np.zeros((128, NB), f32)
    sel_np = np.zeros((128, NB), f32)
    for kk in range(128):
        maskb_np[kk, kk // 16] = 1.0
        if kk // 16 >= 1:
            sel_np[kk, kk // 16 - 1] = 1.0

    smat_pad = np.zeros((128, 128), f32)
    smat_pad[0:16, :] = s_np
    consts_np = np.concatenate([
        cw_tiles, wr_t, wz_t, wn_t, wih2(0), wih2(1), wih2(2), g_np, smat_pad,
        maskb_np, sel_np,
        np.tile(compress_b, 2).reshape(128, 1),
        np.tile(b_ih[:H] + b_hh[:H], NB).reshape(128, 1),
        np.tile(b_ih[H:2 * H] + b_hh[H:2 * H], NB).reshape(128, 1),
        np.tile(b_ih[2 * H:], NB).reshape(128, 1),
        np.tile(b_hh[2 * H:], NB).reshape(128, 1),
    ], axis=1).astype(f32)
    import ml_dtypes
    wbf_np = np.ascontiguousarray(
        np.concatenate([wr_t, wz_t, wn_t], axis=1).astype(ml_dtypes.bfloat16))
    shared = {"consts": np.ascontiguousarray(consts_np), "wbf": wbf_np}

    in_maps = []
    for b in range(NCORES):
        m = dict(shared)
        m["resT"] = np.ascontiguousarray(residual[b].T)
        in_maps.append(m)

    if _NC_CACHE is None:
        _NC_CACHE = _build_nc()
    nc = _NC_CACHE

    res = run_bass_kernel_spmd(nc, in_maps, core_ids=list(range(NCORES)))
    if res.exec_time_ns is not None:
        print(f"HW exec time: {res.exec_time_ns} ns")

    out = np.zeros((B, S, H), f32)
    for b in range(NCORES):
        hb = res.results[b]["out"]                     # [128, L] blocked
        out[b] = hb.reshape(NB, H, L).transpose(0, 2, 1).reshape(S, H)
    return out
